# revision 10
# baseline (speedup 1.0000x reference)
"""Mamba block + FFN fused Trainium2 kernel, 8 NeuronCores.

Sharding: cores 0-3 batch 0, cores 4-7 batch 1. Front half channel-sharded
4-way (512 of d_inner=2048 per core), back half token-sharded (256 of 1024
tokens per core) after an AllToAll. State-major scan (8ch x 16 states per
128-partition tile), G8 shared-output collectives.

v5 vs v2: x tile loads issued before weight loads (LN1 starts ~10us
earlier); gating uses a fused scalar_tensor_tensor; all scan C-multiplies
moved to the Pool engine (DVE keeps only b_t + the scan op, its true
critical path); A2A staging DMAs issue
from the ACT queue (a yg-gated DMA on SP blocked the next group's dur
broadcasts at the SP queue head); yh0/yh1 loads issue from the Pool queue
so the 20MB tail weight stream (w_out/w1/w2) transfers during the AllToAll
window instead of after it; deeper dur/w_out/w1/w2 lookahead buffers.
Alternative restructures (channel-major scan, G4 collectives, per-group
ReduceScatters, exp power-chain) were implemented and measured slower on
hardware; state-major + G8 A2A retained.
"""

import numpy as np
import ml_dtypes

import concourse.bass as bass
import concourse.mybir as mybir
import concourse.tile as tile
from concourse import bacc
from concourse import bass_utils

BF16 = ml_dtypes.bfloat16
F32 = mybir.dt.float32
BF = mybir.dt.bfloat16
AF = mybir.ActivationFunctionType
OP = mybir.AluOpType

B, L, DM = 2, 1024, 1024
DI, DS, DC, DTR, DFF = 2048, 16, 4, 64, 4096
NG = 4
CSH = DI // NG
TSH = L // NG
NTT = 8
EPS = 1e-5
G8 = [[0, 1, 2, 3, 4, 5, 6, 7]]
G4 = [[0, 1, 2, 3], [4, 5, 6, 7]]
NXP = DTR + 2 * DS


def build_kernel(debug_taps=(), reps=1):
    nc = bacc.Bacc("TRN2", target_bir_lowering=False, debug=False,
                   num_devices=8, enable_asserts=False)

    def din(name, shape, dt=F32):
        return nc.dram_tensor(name, shape, dt, kind="ExternalInput").ap()

    x_in = din("x_in", [L, DM], BF)
    xsl = din("xsl", [TSH, DM])
    mk0 = din("mk0", [128, 1])
    mk1 = din("mk1", [128, 1])
    w_in = din("w_in", [DM, 2 * CSH], BF)
    dconv = din("dconv", [CSH, DC * 128], BF)
    w_xp = din("w_xp", [CSH, NXP], BF)
    w_dt = din("w_dt", [DTR, CSH], BF)
    b_dt = din("b_dt", [CSH, 1])
    a_pp = din("a_pp", [128, CSH // 8])
    d_pp = din("d_pp", [128, 4])
    sel = din("sel", [128, 16 * 128], BF)
    selt = din("selt", [128, 16 * 128], BF)
    w_out = din("w_out", [DI, DM], BF)
    w1 = din("w1", [32, 128, 8 * 128], BF)
    w2 = din("w2", [DFF, DM], BF)

    out_ext = nc.dram_tensor("out", [TSH, DM], F32, kind="ExternalOutput").ap()
    taps = {}
    for name, shape, dt in debug_taps:
        taps[name] = nc.dram_tensor("tap_" + name, shape, dt,
                                    kind="ExternalOutput").ap()

    with tile.TileContext(nc) as tc:
        if reps == 1:
            _body(nc, tc, x_in, xsl, w_in, dconv, w_xp, w_dt, b_dt, a_pp, d_pp,
                  sel, selt, w_out, w1, w2, mk0, mk1, out_ext, taps)
        else:
            for _rep in range(reps):
                _body(nc, tc, x_in, xsl, w_in, dconv, w_xp, w_dt, b_dt, a_pp,
                      d_pp, sel, selt, w_out, w1, w2, mk0, mk1, out_ext, taps)
    nc.compile()
    return nc


def _body(nc, tc, x_in, xsl, w_in, dconv, w_xp, w_dt, b_dt, a_pp, d_pp, sel,
          selt, w_out, w1, w2, mk0, mk1, out_ext, taps):
    from contextlib import ExitStack
    es = ExitStack()
    es_a = ExitStack()
    es_b = ExitStack()
    es_c = ExitStack()
    es_d = ExitStack()
    es_e = ExitStack()
    const = es.enter_context(tc.tile_pool(name="const", bufs=1))
    psum = es.enter_context(tc.tile_pool(name="psum", bufs=2, space="PSUM"))
    work = es.enter_context(tc.tile_pool(name="work", bufs=3))
    workb = es.enter_context(tc.tile_pool(name="workb", bufs=4))
    works = es.enter_context(tc.tile_pool(name="works", bufs=6))
    dram = es.enter_context(tc.tile_pool(name="dram", bufs=1, space="DRAM"))
    psy_pool = es_c.enter_context(tc.tile_pool(name="psy", bufs=2, space="PSUM"))
    dtp_pool = es_c.enter_context(tc.tile_pool(name="dtp", bufs=2, space="PSUM"))
    scanp = es_c.enter_context(tc.tile_pool(name="scan", bufs=2))
    scand = es_c.enter_context(tc.tile_pool(name="scand", bufs=4))
    xmbp = es_c.enter_context(tc.tile_pool(name="xmbp", bufs=1))
    poolC = es_c.enter_context(tc.tile_pool(name="poolC", bufs=1))
    poolB = es_b.enter_context(tc.tile_pool(name="poolB", bufs=1))
    poolA = es_a.enter_context(tc.tile_pool(name="poolA", bufs=1))

    # x tiles first on the DMA pipe so LN starts immediately
    xn_dram = dram.tile([L, DM], BF)
    xnT = poolA.tile([128, 8, L], BF)
    x_ts = []
    for i in range(NTT):
        x_t = work.tile([128, DM], BF, tag="xtw")
        nc.sync.dma_start(x_t[:], x_in[i * 128:(i + 1) * 128, :])
        x_ts.append(x_t)

    a_sb = const.tile([128, CSH // 8], F32)
    nc.sync.dma_start(a_sb[:], a_pp[:])
    d_sb = const.tile([128, 4], F32)
    nc.sync.dma_start(d_sb[:], d_pp[:])
    sel_sb = const.tile([128, 16 * 128], BF)
    nc.sync.dma_start(sel_sb[:], sel[:])
    selt_sb = const.tile([128, 16 * 128], BF)
    nc.sync.dma_start(selt_sb[:], selt[:])
    bdt_sb = const.tile([128, 4], F32)
    nc.sync.dma_start(bdt_sb[:], b_dt.rearrange("(m p) o -> p (m o)", p=128))
    eps_sb = const.tile([128, 1], F32)
    nc.gpsimd.memset(eps_sb[:], EPS)
    mk0_sb = const.tile([128, 1], F32)
    nc.sync.dma_start(mk0_sb[:], mk0[:])
    mk1_sb = const.tile([128, 1], F32)
    nc.sync.dma_start(mk1_sb[:], mk1[:])

    win_sb = poolA.tile([128, 8, 2 * CSH], BF)
    nc.sync.dma_start(win_sb[:], w_in.rearrange("(k p) e -> p k e", p=128))
    dconv_sb = poolB.tile([128, 4, DC * 128], BF)
    nc.sync.dma_start(dconv_sb[:], dconv.rearrange("(g p) e -> p g e", p=128))
    wxp_sb = poolC.tile([128, 4, NXP], BF)
    nc.sync.dma_start(wxp_sb[:], w_xp.rearrange("(k p) e -> p k e", p=128))
    wdt_sb = poolC.tile([64, DTR * CSH // 64], BF)
    nc.sync.dma_start(wdt_sb[:], w_dt[:, :])

    for i in range(NTT):
        x_t = x_ts[i]
        st6 = works.tile([128, 12], F32, tag="sm")
        nc.vector.bn_stats(st6[:, 0:6], x_t[:, 0:512])
        nc.vector.bn_stats(st6[:, 6:12], x_t[:, 512:1024])
        ag = works.tile([128, 2], F32, tag="sm2")
        nc.vector.bn_aggr(ag[:], st6[:])
        svar = works.tile([128, 1], F32, tag="sm3")
        nc.scalar.activation(svar[:], ag[:, 1:2], AF.Sqrt, bias=eps_sb[:])
        rstd = works.tile([128, 1], F32, tag="sm4")
        nc.vector.reciprocal(rstd[:], svar[:])
        xn = workb.tile([128, DM], BF, tag="bfw")
        nc.vector.tensor_scalar(xn[:], x_t[:], ag[:, 0:1], rstd[:],
                                OP.subtract, OP.mult)
        nc.sync.dma_start(xn_dram[i * 128:(i + 1) * 128, :], xn[:])
        nc.sync.dma_start_transpose(
            xnT[:, :, i * 128:(i + 1) * 128],
            xn_dram[i * 128:(i + 1) * 128, :])

    u0 = poolB.tile([128, 4, DC - 1 + L], BF)
    z0 = poolB.tile([128, 4, L], BF)
    for g in range(4):
        nc.gpsimd.memset(u0[:, g, 0:DC - 1], 0.0)
    for tb in range(2):
        for m in range(8):
            ps = psum.tile([128, 512], F32, tag="pmm")
            for k in range(8):
                nc.tensor.matmul(ps[:], win_sb[:, k, m * 128:(m + 1) * 128],
                                 xnT[:, k, tb * 512:(tb + 1) * 512],
                                 start=(k == 0), stop=(k == 7))
            if m < 4:
                dst = u0[:, m, DC - 1 + tb * 512: DC - 1 + (tb + 1) * 512]
                if tb == 0:
                    nc.scalar.copy(dst, ps[:])
                else:
                    nc.vector.tensor_copy(dst, ps[:])
            else:
                nc.vector.tensor_copy(z0[:, m - 4, tb * 512:(tb + 1) * 512],
                                      ps[:])

    es_a.close()

    u_bf = poolC.tile([128, 4, L], BF)
    for tb in range(2):
        for g in range(4):
            ps = psum.tile([128, 512], F32, tag="pmm")
            for k in range(DC):
                nc.tensor.matmul(ps[:], dconv_sb[:, g, k * 128:(k + 1) * 128],
                                 u0[:, g, tb * 512 + k: tb * 512 + k + 512],
                                 start=(k == 0), stop=(k == DC - 1))
            nc.scalar.activation(u_bf[:, g, tb * 512:(tb + 1) * 512], ps[:],
                                 AF.Silu)
    z_s = poolC.tile([128, 4, L], BF)
    for g in range(4):
        nc.scalar.activation(z_s[:, g, :], z0[:, g, :], AF.Silu)

    es_b.close()

    xdbp = workb.tile([96, L], BF, tag="xpb")
    for tb in range(2):
        ps = psum.tile([96, 512], F32, tag="pmm")
        for k in range(4):
            nc.tensor.matmul(ps[:], wxp_sb[:, k, :],
                             u_bf[:, k, tb * 512:(tb + 1) * 512],
                             start=(k == 0), stop=(k == 3))
        nc.vector.tensor_copy(xdbp[:, tb * 512:(tb + 1) * 512], ps[:])
    xdb_in = dram.tile([192, L], BF)
    xdb_out = nc.dram_tensor(f"xdb_out_sh_{nc.next_id()}", [192, L], BF,
                             kind="Internal", addr_space="Shared").ap()
    xmb = xmbp.tile([96, 2, L], BF, tag="xmb")
    nc.vector.tensor_scalar(xmb[:, 0, :], xdbp[:], mk0_sb[0:96, :], None, OP.mult)
    nc.vector.tensor_scalar(xmb[:, 1, :], xdbp[:], mk1_sb[0:96, :], None, OP.mult)
    nc.gpsimd.dma_start(
        xdb_in[:].rearrange("(s r) t -> r s t", s=2), xmb[:])
    nc.gpsimd.collective_compute(
        "AllReduce", OP.add, replica_groups=G8,
        ins=[xdb_in[:].opt()], outs=[xdb_out[:].opt()])
    s0 = workb.tile([96, L], BF, tag="xpb")
    nc.sync.dma_start(s0[:], xdb_out[0:96, :])
    s1 = workb.tile([96, L], BF, tag="xpb")
    nc.sync.dma_start(s1[:], xdb_out[96:192, :])
    xdb_bf = poolC.tile([96, L], BF)
    nc.vector.tensor_scalar(xdb_bf[:], s0[:], mk0_sb[0:96, :], None, OP.mult)
    nc.vector.scalar_tensor_tensor(xdb_bf[:], s1[:], mk1_sb[0:96, :], xdb_bf[:],
                                   OP.mult, OP.add)
    xdbc_dram = dram.tile([96, L], BF)
    nc.sync.dma_start(xdbc_dram[:], xdb_bf[:])
    if "xdb" in taps:
        nc.sync.dma_start(taps["xdb"][:], xdb_bf[:])
    brep = poolC.tile([128, L], BF)
    crep = poolC.tile([128, L], BF)
    nc.sync.dma_start(
        brep[:], xdbc_dram[DTR:DTR + DS, None, :].to_broadcast((DS, 8, L)))
    nc.sync.dma_start(
        crep[:], xdbc_dram[DTR + DS:DTR + 2 * DS, None, :].to_broadcast((DS, 8, L)))

    dt_bf = poolC.tile([128, 4, L], BF)
    dtu_bf = poolC.tile([128, 4, L], BF)
    et_all = poolC.tile([128, 4, L], F32)
    for m in range(4):
        for tb in range(2):
            ps = psum.tile([128, 512], F32, tag="pmm")
            nc.tensor.matmul(ps[:], wdt_sb[:, m * 128:(m + 1) * 128],
                             xdb_bf[0:64, tb * 512:(tb + 1) * 512],
                             start=True, stop=True)
            nc.scalar.activation(et_all[:, m, tb * 512:(tb + 1) * 512], ps[:],
                                 AF.Exp, bias=bdt_sb[:, m:m + 1])
    for m in range(4):
        nc.scalar.activation(dt_bf[:, m, :], et_all[:, m, :], AF.Ln, bias=1.0)
        nc.vector.tensor_tensor(dtu_bf[:, m, :], dt_bf[:, m, :], u_bf[:, m, :],
                                OP.mult)
    ddtu = dram.tile([CSH, L], BF)
    for m in range(4):
        nc.sync.dma_start(ddtu[m * 128:(m + 1) * 128, :], dtu_bf[:, m, :])

    a2a_in = dram.tile([2 * DI, TSH], BF)
    a2a_out = dram.tile([2 * DI, TSH], BF)
    for g in range(4):
        psy = psy_pool.tile([128, L], F32, tag="psy")
        for q in range(16):
            ct = g * 16 + q
            dA = scanp.tile([128, L], BF, tag="dA")
            for tb in range(2):
                dtp = dtp_pool.tile([128, 512], F32, tag="dtp")
                nc.tensor.matmul(dtp[:], selt_sb[:, q * 128:(q + 1) * 128],
                                 dt_bf[:, g, tb * 512:(tb + 1) * 512],
                                 start=True, stop=True)
                nc.scalar.activation(dA[:, tb * 512:(tb + 1) * 512], dtp[:],
                                     AF.Exp, scale=a_sb[:, ct:ct + 1])
            dur = scand.tile([128, L], BF, tag="dur")
            nc.sync.dma_start(
                dur[:], ddtu[None, ct * 8:(ct + 1) * 8, :].to_broadcast((16, 8, L)))
            b_t = scanp.tile([128, L], BF, tag="bt")
            nc.vector.tensor_tensor(b_t[:], dur[:], brep[:], OP.mult)
            h_t = scanp.tile([128, L], BF, tag="ht")
            nc.vector.tensor_tensor_scan(h_t[:], dA[:], b_t[:], 0.0,
                                         OP.mult, OP.add)
            ch_t = scanp.tile([128, L], BF, tag="cht")
            nc.gpsimd.tensor_tensor(ch_t[:], h_t[:], crep[:], OP.mult)
            for tb in range(2):
                nc.tensor.matmul(psy[:, tb * 512:(tb + 1) * 512],
                                 sel_sb[:, q * 128:(q + 1) * 128],
                                 ch_t[:, tb * 512:(tb + 1) * 512],
                                 start=(q == 0), stop=(q == 15))
        t1 = work.tile([128, L], BF, tag="t1w")
        nc.vector.scalar_tensor_tensor(t1[:], u_bf[:, g, :], d_sb[:, g:g + 1],
                                       psy[:], OP.mult, OP.add)
        yg = workb.tile([128, L], BF, tag="bfw")
        nc.vector.tensor_tensor(yg[:], t1[:], z_s[:, g, :], OP.mult)
        if "yg" in taps:
            nc.sync.dma_start(taps["yg"][g * 128:(g + 1) * 128, :], yg[:])
        a2a_v = a2a_in[:].rearrange("(h j m p) t -> h m p j t", h=2, j=4, m=4)
        ygm = workb.tile([128, L], BF, tag="bfw")
        nc.vector.tensor_scalar(ygm[:], yg[:], mk0_sb[:], None, OP.mult)
        # staging on the ACT queue: a yg-gated DMA on SP would block the
        # next group's dur broadcasts at the SP queue head
        nc.scalar.dma_start(
            a2a_v[0, g], ygm[:].rearrange("p (j t) -> p j t", t=TSH))
        ygm2 = workb.tile([128, L], BF, tag="bfw")
        nc.vector.tensor_scalar(ygm2[:], yg[:], mk1_sb[:], None, OP.mult)
        nc.scalar.dma_start(
            a2a_v[1, g], ygm2[:].rearrange("p (j t) -> p j t", t=TSH))

    nc.gpsimd.collective_compute(
        "AllToAll", OP.bypass, replica_groups=G8,
        ins=[a2a_in[:].opt()], outs=[a2a_out[:].opt()])
    es_c.close()

    poolBK = es.enter_context(tc.tile_pool(name="poolBK", bufs=1))
    poolD = es_d.enter_context(tc.tile_pool(name="poolD", bufs=1))
    # yh loads on the Pool queue (already serialized behind the A2A);
    # keeping them off SP lets the tail weight streams (w_out/w1/w2) issue
    # and transfer during the collective window
    yh0 = poolD.tile([128, 16, TSH], BF)
    nc.gpsimd.dma_start(yh0[:], a2a_out[0:DI, :].rearrange("(k p) t -> p k t", p=128))
    yh1 = poolD.tile([128, 16, TSH], BF)
    nc.gpsimd.dma_start(yh1[:], a2a_out[DI:2 * DI, :].rearrange("(k p) t -> p k t", p=128))
    ygf = poolD.tile([128, 16, TSH], BF)
    nc.vector.tensor_tensor(ygf[:], yh0[:], yh1[:], OP.add)
    xsl_sb = poolBK.tile([128, 2, DM], F32)
    nc.sync.dma_start(xsl_sb[:], xsl.rearrange("(h p) m -> p h m", p=128))
    x2 = poolBK.tile([128, 2, DM], F32)
    with tc.tile_pool(name="wop", bufs=16) as wop:
        pso = {}
        with tc.tile_pool(name="pso", bufs=1, space="PSUM") as pso_pool:
            for th in range(2):
                for ms in range(2):
                    pso[(th, ms)] = pso_pool.tile([128, 512], F32,
                                                  tag=f"pso_{th}_{ms}",
                                                  name=f"pso_{th}_{ms}")
            for k in range(16):
                woutk = wop.tile([128, DM], BF, tag="woutk")
                nc.sync.dma_start(woutk[:], w_out[k * 128:(k + 1) * 128, :])
                for th in range(2):
                    for ms in range(2):
                        nc.tensor.matmul(pso[(th, ms)][:],
                                         ygf[:, k, th * 128:(th + 1) * 128],
                                         woutk[:, ms * 512:(ms + 1) * 512],
                                         start=(k == 0), stop=(k == 15))
            for th in range(2):
                for ms in range(2):
                    nc.vector.tensor_tensor(
                        x2[:, th, ms * 512:(ms + 1) * 512], pso[(th, ms)][:],
                        xsl_sb[:, th, ms * 512:(ms + 1) * 512], OP.add)
    es_d.close()

    x2n_dram = dram.tile([TSH, DM], BF)
    x2nT = poolBK.tile([128, 8, TSH], BF)
    for th in range(2):
        st6 = works.tile([128, 12], F32, tag="sm")
        nc.vector.bn_stats(st6[:, 0:6], x2[:, th, 0:512])
        nc.vector.bn_stats(st6[:, 6:12], x2[:, th, 512:1024])
        ag = works.tile([128, 2], F32, tag="sm2")
        nc.vector.bn_aggr(ag[:], st6[:])
        svar = works.tile([128, 1], F32, tag="sm3")
        nc.scalar.activation(svar[:], ag[:, 1:2], AF.Sqrt, bias=eps_sb[:])
        rstd = works.tile([128, 1], F32, tag="sm4")
        nc.vector.reciprocal(rstd[:], svar[:])
        x2n = workb.tile([128, DM], BF, tag="bfw")
        nc.vector.tensor_scalar(x2n[:], x2[:, th, :], ag[:, 0:1], rstd[:],
                                OP.subtract, OP.mult)
        nc.sync.dma_start(x2n_dram[th * 128:(th + 1) * 128, :], x2n[:])
        nc.sync.dma_start_transpose(
            x2nT[:, :, th * 128:(th + 1) * 128],
            x2n_dram[th * 128:(th + 1) * 128, :])

    h1T = poolBK.tile([128, 32, TSH], BF)
    with tc.tile_pool(name="w1p", bufs=16) as w1p, \
         tc.tile_pool(name="pp1", bufs=2, space="PSUM") as pp1:
        for ff in range(32):
            w1f = w1p.tile([128, 8, 128], BF, tag="w1f")
            nc.sync.dma_start(w1f[:], w1[ff])
            ps = pp1.tile([128, TSH], F32, tag="pf1")
            for k in range(8):
                nc.tensor.matmul(ps[:], w1f[:, k, :],
                                 x2nT[:, k, :],
                                 start=(k == 0), stop=(k == 7))
            nc.scalar.activation(h1T[:, ff, :], ps[:], AF.Relu)
    es_e.close()

    with tc.tile_pool(name="pf2", bufs=1, space="PSUM") as pf2, \
         tc.tile_pool(name="w2p", bufs=8) as w2p:
        pss = {}
        for th in range(2):
            for ms in range(2):
                pss[(th, ms)] = pf2.tile([128, 512], F32, tag=f"po2_{th}_{ms}",
                                         name=f"po2_{th}_{ms}")
        for k in range(32):
            w2k = w2p.tile([128, DM], BF, tag="w2k")
            nc.sync.dma_start(w2k[:], w2[k * 128:(k + 1) * 128, :])
            for th in range(2):
                for ms in range(2):
                    nc.tensor.matmul(pss[(th, ms)][:],
                                     h1T[:, k, th * 128:(th + 1) * 128],
                                     w2k[:, ms * 512:(ms + 1) * 512],
                                     start=(k == 0), stop=(k == 31))
        for th in range(2):
            for ms in range(2):
                ot = work.tile([128, 512], F32, tag="f32w")
                nc.vector.tensor_tensor(ot[:], pss[(th, ms)][:],
                                        x2[:, th, ms * 512:(ms + 1) * 512], OP.add)
                nc.sync.dma_start(out_ext[th * 128:(th + 1) * 128,
                                          ms * 512:(ms + 1) * 512], ot[:])

    es.close()


def _prep_core_inputs(inputs):
    x = np.asarray(inputs["x"], np.float32)
    W_in = np.asarray(inputs["W_in"], np.float32)
    conv_w = np.asarray(inputs["conv_w"], np.float32)
    W_xp = np.asarray(inputs["W_xproj"], np.float32)
    W_dt = np.asarray(inputs["W_dt"], np.float32)
    b_dt = np.asarray(inputs["b_dt"], np.float32)
    A_log = np.asarray(inputs["A_log"], np.float32)
    D = np.asarray(inputs["D"], np.float32)
    W_out = np.asarray(inputs["W_out"], np.float32)
    W1 = np.asarray(inputs["W1"], np.float32)
    W2 = np.asarray(inputs["W2"], np.float32)
    ln1_w = np.asarray(inputs["ln1_w"], np.float32)
    ln1_b = np.asarray(inputs["ln1_b"], np.float32)
    ln2_w = np.asarray(inputs["ln2_w"], np.float32)
    ln2_b = np.asarray(inputs["ln2_b"], np.float32)
    b1 = np.asarray(inputs["b1"], np.float32)
    b2 = np.asarray(inputs["b2"], np.float32)
    conv_b = np.asarray(inputs["conv_b"], np.float32)

    W_in = W_in * ln1_w[None, :]
    W1 = W1 * ln2_w[None, :]
    assert np.allclose(ln1_b, 0.0) and np.allclose(ln2_b, 0.0)
    assert np.allclose(b1, 0.0) and np.allclose(b2, 0.0)
    assert np.allclose(conv_b, 0.0)

    A = -np.exp(A_log)

    in_maps = []
    for core in range(8):
        g, r = core // NG, core % NG
        ch = slice(r * CSH, (r + 1) * CSH)
        m = {}
        m["x_in"] = np.ascontiguousarray(x[g]).astype(BF16)
        m["xsl"] = np.ascontiguousarray(x[g][r * TSH:(r + 1) * TSH, :])
        wu = W_in[ch, :]
        wz = W_in[DI + r * CSH: DI + (r + 1) * CSH, :]
        m["w_in"] = np.ascontiguousarray(
            np.concatenate([wu.T, wz.T], axis=1).astype(BF16))
        dg = np.zeros((CSH, DC * 128), np.float32)
        cw = conv_w[ch, :]
        for gg in range(4):
            for c in range(128):
                for k in range(DC):
                    dg[gg * 128 + c, k * 128 + c] = cw[gg * 128 + c, k]
        m["dconv"] = dg.astype(BF16)
        m["w_xp"] = np.ascontiguousarray(W_xp[:, ch].T).astype(BF16)
        m["w_dt"] = np.ascontiguousarray(W_dt[ch, :].T).astype(BF16)
        m["b_dt"] = np.ascontiguousarray(b_dt[ch, None])
        app = np.zeros((128, CSH // 8), np.float32)
        for ct in range(CSH // 8):
            for p in range(128):
                s, d = p // 8, p % 8
                app[p, ct] = A[r * CSH + ct * 8 + d, s]
        m["a_pp"] = app
        dpp = np.zeros((128, 4), np.float32)
        for gg in range(4):
            dpp[:, gg] = D[r * CSH + gg * 128: r * CSH + (gg + 1) * 128]
        m["d_pp"] = dpp
        selm = np.zeros((128, 16 * 128), np.float32)
        seltm = np.zeros((128, 16 * 128), np.float32)
        for q in range(16):
            for p in range(128):
                selm[p, q * 128 + q * 8 + (p % 8)] = 1.0
                seltm[q * 8 + (p % 8), q * 128 + p] = 1.0
        m["sel"] = selm.astype(BF16)
        m["selt"] = seltm.astype(BF16)
        m["w_out"] = np.ascontiguousarray(W_out.T.astype(BF16))
        m["mk0"] = np.full((128, 1), 1.0 if g == 0 else 0.0, np.float32)
        m["mk1"] = np.full((128, 1), 1.0 if g == 1 else 0.0, np.float32)
        w1r = W1.reshape(32, 128, 8, 128)
        w1r = np.transpose(w1r, (0, 3, 2, 1))
        m["w1"] = np.ascontiguousarray(w1r.reshape(32, 128, 8 * 128).astype(BF16))
        m["w2"] = np.ascontiguousarray(W2.T.astype(BF16))
        in_maps.append(m)
    return in_maps


_NC = None


def kernel(**inputs):
    global _NC
    if _NC is None:
        _NC = build_kernel()
    in_maps = _prep_core_inputs(inputs)
    res = bass_utils.run_bass_kernel_spmd(_NC, in_maps, core_ids=list(range(8)))
    out = np.zeros((B, L, DM), np.float32)
    for core in range(8):
        g, r = core // NG, core % NG
        out[g, r * TSH:(r + 1) * TSH, :] = res.results[core]["out"]
    return out


# revision 13
# speedup vs baseline: 1.0205x; 1.0205x over previous
"""Mamba block + FFN fused Trainium2 kernel, 8 NeuronCores.

Sharding: cores 0-3 batch 0, cores 4-7 batch 1. Front half channel-sharded
4-way (512 of d_inner=2048 per core), back half token-sharded (256 of 1024
tokens per core) after an AllToAll. State-major scan (8ch x 16 states per
128-partition tile), G8 shared-output collectives.

v5 vs v2: x tile loads issued before weight loads (LN1 starts ~10us
earlier); gating uses a fused scalar_tensor_tensor; all scan C-multiplies
moved to the Pool engine (DVE keeps only b_t + the scan op, its true
critical path); A2A staging DMAs issue
from the ACT queue (a yg-gated DMA on SP blocked the next group's dur
broadcasts at the SP queue head); yh0/yh1 loads issue from the Pool queue
so the 20MB tail weight stream (w_out/w1/w2) transfers during the AllToAll
window instead of after it; deeper dur/w_out/w1/w2 lookahead buffers.
Alternative restructures (channel-major scan, G4 collectives, per-group
ReduceScatters, exp power-chain) were implemented and measured slower on
hardware; state-major + G8 A2A retained.
"""

import numpy as np
import ml_dtypes

import concourse.bass as bass
import concourse.mybir as mybir
import concourse.tile as tile
from concourse import bacc
from concourse import bass_utils

BF16 = ml_dtypes.bfloat16
F32 = mybir.dt.float32
BF = mybir.dt.bfloat16
AF = mybir.ActivationFunctionType
OP = mybir.AluOpType

B, L, DM = 2, 1024, 1024
DI, DS, DC, DTR, DFF = 2048, 16, 4, 64, 4096
NG = 4
CSH = DI // NG
TSH = L // NG
NTT = 8
EPS = 1e-5
G8 = [[0, 1, 2, 3, 4, 5, 6, 7]]
G4 = [[0, 1, 2, 3], [4, 5, 6, 7]]
NXP = DTR + 2 * DS


def build_kernel(debug_taps=(), reps=1):
    nc = bacc.Bacc("TRN2", target_bir_lowering=False, debug=False,
                   num_devices=8, enable_asserts=False)

    def din(name, shape, dt=F32):
        return nc.dram_tensor(name, shape, dt, kind="ExternalInput").ap()

    x_in = din("x_in", [L, DM], BF)
    xsl = din("xsl", [TSH, DM])
    mk0 = din("mk0", [128, 1])
    mk1 = din("mk1", [128, 1])
    w_in = din("w_in", [DM, 2 * CSH], BF)
    dconv = din("dconv", [CSH, DC * 128], BF)
    w_xp = din("w_xp", [CSH, NXP], BF)
    w_dt = din("w_dt", [DTR, CSH], BF)
    b_dt = din("b_dt", [CSH, 1])
    a_pp = din("a_pp", [128, CSH // 8])
    d_pp = din("d_pp", [128, 4])
    sel = din("sel", [128, 16 * 128], BF)
    selt = din("selt", [128, 16 * 128], BF)
    w_out = din("w_out", [DI, DM], BF)
    w1 = din("w1", [32, 128, 8 * 128], BF)
    w2 = din("w2", [DFF, DM], BF)

    out_ext = nc.dram_tensor("out", [TSH, DM], F32, kind="ExternalOutput").ap()
    taps = {}
    for name, shape, dt in debug_taps:
        taps[name] = nc.dram_tensor("tap_" + name, shape, dt,
                                    kind="ExternalOutput").ap()

    with tile.TileContext(nc) as tc:
        if reps == 1:
            _body(nc, tc, x_in, xsl, w_in, dconv, w_xp, w_dt, b_dt, a_pp, d_pp,
                  sel, selt, w_out, w1, w2, mk0, mk1, out_ext, taps)
        else:
            for _rep in range(reps):
                _body(nc, tc, x_in, xsl, w_in, dconv, w_xp, w_dt, b_dt, a_pp,
                      d_pp, sel, selt, w_out, w1, w2, mk0, mk1, out_ext, taps)
    nc.compile()
    return nc


def _body(nc, tc, x_in, xsl, w_in, dconv, w_xp, w_dt, b_dt, a_pp, d_pp, sel,
          selt, w_out, w1, w2, mk0, mk1, out_ext, taps):
    from contextlib import ExitStack
    es = ExitStack()
    es_a = ExitStack()
    es_b = ExitStack()
    es_c = ExitStack()
    es_d = ExitStack()
    es_e = ExitStack()
    const = es.enter_context(tc.tile_pool(name="const", bufs=1))
    psum = es.enter_context(tc.tile_pool(name="psum", bufs=2, space="PSUM"))
    work = es.enter_context(tc.tile_pool(name="work", bufs=3))
    workb = es.enter_context(tc.tile_pool(name="workb", bufs=4))
    works = es.enter_context(tc.tile_pool(name="works", bufs=6))
    dram = es.enter_context(tc.tile_pool(name="dram", bufs=1, space="DRAM"))
    psy_pool = es_c.enter_context(tc.tile_pool(name="psy", bufs=2, space="PSUM"))
    dtp_pool = es_c.enter_context(tc.tile_pool(name="dtp", bufs=2, space="PSUM"))
    scanp = es_c.enter_context(tc.tile_pool(name="scan", bufs=2))
    scand = es_c.enter_context(tc.tile_pool(name="scand", bufs=4))
    xmbp = es_c.enter_context(tc.tile_pool(name="xmbp", bufs=1))
    poolC = es_c.enter_context(tc.tile_pool(name="poolC", bufs=1))
    poolB = es_b.enter_context(tc.tile_pool(name="poolB", bufs=1))
    poolA = es_a.enter_context(tc.tile_pool(name="poolA", bufs=1))

    # x tiles first on the DMA pipe so LN starts immediately
    xn_dram = dram.tile([L, DM], BF)
    xnT = poolA.tile([128, 8, L], BF)
    x_ts = []
    for i in range(NTT):
        x_t = work.tile([128, DM], BF, tag="xtw")
        nc.sync.dma_start(x_t[:], x_in[i * 128:(i + 1) * 128, :])
        x_ts.append(x_t)

    a_sb = const.tile([128, CSH // 8], F32)
    nc.sync.dma_start(a_sb[:], a_pp[:])
    d_sb = const.tile([128, 4], F32)
    nc.sync.dma_start(d_sb[:], d_pp[:])
    sel_sb = const.tile([128, 16 * 128], BF)
    nc.sync.dma_start(sel_sb[:], sel[:])
    selt_sb = const.tile([128, 16 * 128], BF)
    nc.sync.dma_start(selt_sb[:], selt[:])
    bdt_sb = const.tile([128, 4], F32)
    nc.sync.dma_start(bdt_sb[:], b_dt.rearrange("(m p) o -> p (m o)", p=128))
    eps_sb = const.tile([128, 1], F32)
    nc.gpsimd.memset(eps_sb[:], EPS)
    mk0_sb = const.tile([128, 1], F32)
    nc.sync.dma_start(mk0_sb[:], mk0[:])
    mk1_sb = const.tile([128, 1], F32)
    nc.sync.dma_start(mk1_sb[:], mk1[:])

    win_sb = poolA.tile([128, 8, 2 * CSH], BF)
    nc.sync.dma_start(win_sb[:], w_in.rearrange("(k p) e -> p k e", p=128))
    dconv_sb = poolB.tile([128, 4, DC * 128], BF)
    nc.sync.dma_start(dconv_sb[:], dconv.rearrange("(g p) e -> p g e", p=128))
    wxp_sb = poolC.tile([128, 4, NXP], BF)
    nc.sync.dma_start(wxp_sb[:], w_xp.rearrange("(k p) e -> p k e", p=128))
    wdt_sb = poolC.tile([64, DTR * CSH // 64], BF)
    nc.sync.dma_start(wdt_sb[:], w_dt[:, :])

    for i in range(NTT):
        x_t = x_ts[i]
        st6 = works.tile([128, 12], F32, tag="sm")
        nc.vector.bn_stats(st6[:, 0:6], x_t[:, 0:512])
        nc.vector.bn_stats(st6[:, 6:12], x_t[:, 512:1024])
        ag = works.tile([128, 2], F32, tag="sm2")
        nc.vector.bn_aggr(ag[:], st6[:])
        svar = works.tile([128, 1], F32, tag="sm3")
        nc.scalar.activation(svar[:], ag[:, 1:2], AF.Sqrt, bias=eps_sb[:])
        rstd = works.tile([128, 1], F32, tag="sm4")
        nc.vector.reciprocal(rstd[:], svar[:])
        xn = workb.tile([128, DM], BF, tag="bfw")
        nc.vector.tensor_scalar(xn[:], x_t[:], ag[:, 0:1], rstd[:],
                                OP.subtract, OP.mult)
        nc.sync.dma_start(xn_dram[i * 128:(i + 1) * 128, :], xn[:])
        nc.sync.dma_start_transpose(
            xnT[:, :, i * 128:(i + 1) * 128],
            xn_dram[i * 128:(i + 1) * 128, :])

    u0 = poolB.tile([128, 4, DC - 1 + L], BF)
    z0 = poolB.tile([128, 4, L], BF)
    for g in range(4):
        nc.gpsimd.memset(u0[:, g, 0:DC - 1], 0.0)
    for tb in range(2):
        for m in range(8):
            ps = psum.tile([128, 512], F32, tag="pmm")
            for k in range(8):
                nc.tensor.matmul(ps[:], win_sb[:, k, m * 128:(m + 1) * 128],
                                 xnT[:, k, tb * 512:(tb + 1) * 512],
                                 start=(k == 0), stop=(k == 7))
            if m < 4:
                dst = u0[:, m, DC - 1 + tb * 512: DC - 1 + (tb + 1) * 512]
                if tb == 0:
                    nc.scalar.copy(dst, ps[:])
                else:
                    nc.vector.tensor_copy(dst, ps[:])
            else:
                nc.vector.tensor_copy(z0[:, m - 4, tb * 512:(tb + 1) * 512],
                                      ps[:])

    es_a.close()

    u_bf = poolC.tile([128, 4, L], BF)
    for tb in range(2):
        for g in range(4):
            ps = psum.tile([128, 512], F32, tag="pmm")
            for k in range(DC):
                nc.tensor.matmul(ps[:], dconv_sb[:, g, k * 128:(k + 1) * 128],
                                 u0[:, g, tb * 512 + k: tb * 512 + k + 512],
                                 start=(k == 0), stop=(k == DC - 1))
            nc.scalar.activation(u_bf[:, g, tb * 512:(tb + 1) * 512], ps[:],
                                 AF.Silu)
    z_s = poolC.tile([128, 4, L], BF)
    for g in range(4):
        nc.scalar.activation(z_s[:, g, :], z0[:, g, :], AF.Silu)

    es_b.close()

    xdbp = workb.tile([96, L], BF, tag="xpb")
    for tb in range(2):
        ps = psum.tile([96, 512], F32, tag="pmm")
        for k in range(4):
            nc.tensor.matmul(ps[:], wxp_sb[:, k, :],
                             u_bf[:, k, tb * 512:(tb + 1) * 512],
                             start=(k == 0), stop=(k == 3))
        nc.vector.tensor_copy(xdbp[:, tb * 512:(tb + 1) * 512], ps[:])
    xdb_in = dram.tile([192, L], BF)
    xdb_out = nc.dram_tensor(f"xdb_out_sh_{nc.next_id()}", [192, L], BF,
                             kind="Internal", addr_space="Shared").ap()
    xmb = xmbp.tile([96, 2, L], BF, tag="xmb")
    nc.vector.tensor_scalar(xmb[:, 0, :], xdbp[:], mk0_sb[0:96, :], None, OP.mult)
    nc.vector.tensor_scalar(xmb[:, 1, :], xdbp[:], mk1_sb[0:96, :], None, OP.mult)
    nc.gpsimd.dma_start(
        xdb_in[:].rearrange("(s r) t -> r s t", s=2), xmb[:])
    nc.gpsimd.collective_compute(
        "AllReduce", OP.add, replica_groups=G8,
        ins=[xdb_in[:].opt()], outs=[xdb_out[:].opt()])
    s0 = workb.tile([96, L], BF, tag="xpb")
    nc.sync.dma_start(s0[:], xdb_out[0:96, :])
    s1 = workb.tile([96, L], BF, tag="xpb")
    nc.sync.dma_start(s1[:], xdb_out[96:192, :])
    xdb_bf = poolC.tile([96, L], BF)
    nc.vector.tensor_scalar(xdb_bf[:], s0[:], mk0_sb[0:96, :], None, OP.mult)
    nc.vector.scalar_tensor_tensor(xdb_bf[:], s1[:], mk1_sb[0:96, :], xdb_bf[:],
                                   OP.mult, OP.add)
    xdbc_dram = dram.tile([96, L], BF)
    nc.sync.dma_start(xdbc_dram[:], xdb_bf[:])
    if "xdb" in taps:
        nc.sync.dma_start(taps["xdb"][:], xdb_bf[:])
    brep = poolC.tile([128, L], BF)
    crep = poolC.tile([128, L], BF)
    nc.sync.dma_start(
        brep[:], xdbc_dram[DTR:DTR + DS, None, :].to_broadcast((DS, 8, L)))
    nc.sync.dma_start(
        crep[:], xdbc_dram[DTR + DS:DTR + 2 * DS, None, :].to_broadcast((DS, 8, L)))

    dt_bf = poolC.tile([128, 4, L], BF)
    dtu_bf = poolC.tile([128, 4, L], BF)
    et_all = poolC.tile([128, 4, L], F32)
    for m in range(4):
        for tb in range(2):
            ps = psum.tile([128, 512], F32, tag="pmm")
            nc.tensor.matmul(ps[:], wdt_sb[:, m * 128:(m + 1) * 128],
                             xdb_bf[0:64, tb * 512:(tb + 1) * 512],
                             start=True, stop=True)
            nc.scalar.activation(et_all[:, m, tb * 512:(tb + 1) * 512], ps[:],
                                 AF.Exp, bias=bdt_sb[:, m:m + 1])
    for m in range(4):
        nc.scalar.activation(dt_bf[:, m, :], et_all[:, m, :], AF.Ln, bias=1.0)
        nc.vector.tensor_tensor(dtu_bf[:, m, :], dt_bf[:, m, :], u_bf[:, m, :],
                                OP.mult)
    ddtu = dram.tile([CSH, L], BF)
    for m in range(4):
        nc.sync.dma_start(ddtu[m * 128:(m + 1) * 128, :], dtu_bf[:, m, :])

    a2a_in = dram.tile([2 * DI, TSH], BF)
    a2a_out = dram.tile([2 * DI, TSH], BF)
    for g in range(4):
        psy = psy_pool.tile([128, L], F32, tag="psy")
        for q in range(16):
            ct = g * 16 + q
            dA = scanp.tile([128, L], BF, tag="dA")
            for tb in range(2):
                dtp = dtp_pool.tile([128, 512], F32, tag="dtp")
                nc.tensor.matmul(dtp[:], selt_sb[:, q * 128:(q + 1) * 128],
                                 dt_bf[:, g, tb * 512:(tb + 1) * 512],
                                 start=True, stop=True)
                nc.scalar.activation(dA[:, tb * 512:(tb + 1) * 512], dtp[:],
                                     AF.Exp, scale=a_sb[:, ct:ct + 1])
            dur = scand.tile([128, L], BF, tag="dur")
            nc.sync.dma_start(
                dur[:], ddtu[None, ct * 8:(ct + 1) * 8, :].to_broadcast((16, 8, L)))
            b_t = scanp.tile([128, L], BF, tag="bt")
            nc.vector.tensor_tensor(b_t[:], dur[:], brep[:], OP.mult)
            h_t = scanp.tile([128, L], BF, tag="ht")
            nc.vector.tensor_tensor_scan(h_t[:], dA[:], b_t[:], 0.0,
                                         OP.mult, OP.add)
            ch_t = scanp.tile([128, L], BF, tag="cht")
            nc.gpsimd.tensor_tensor(ch_t[:], h_t[:], crep[:], OP.mult)
            for tb in range(2):
                nc.tensor.matmul(psy[:, tb * 512:(tb + 1) * 512],
                                 sel_sb[:, q * 128:(q + 1) * 128],
                                 ch_t[:, tb * 512:(tb + 1) * 512],
                                 start=(q == 0), stop=(q == 15))
        t1 = work.tile([128, L], BF, tag="t1w")
        nc.vector.scalar_tensor_tensor(t1[:], u_bf[:, g, :], d_sb[:, g:g + 1],
                                       psy[:], OP.mult, OP.add)
        yg = workb.tile([128, L], BF, tag="bfw")
        nc.gpsimd.tensor_tensor(yg[:], t1[:], z_s[:, g, :], OP.mult)
        if "yg" in taps:
            nc.sync.dma_start(taps["yg"][g * 128:(g + 1) * 128, :], yg[:])
        a2a_v = a2a_in[:].rearrange("(h j m p) t -> h m p j t", h=2, j=4, m=4)
        ygm = workb.tile([128, L], BF, tag="bfw")
        nc.vector.tensor_scalar(ygm[:], yg[:], mk0_sb[:], None, OP.mult)
        # staging on the ACT queue: a yg-gated DMA on SP would block the
        # next group's dur broadcasts at the SP queue head
        nc.scalar.dma_start(
            a2a_v[0, g], ygm[:].rearrange("p (j t) -> p j t", t=TSH))
        ygm2 = workb.tile([128, L], BF, tag="bfw")
        nc.vector.tensor_scalar(ygm2[:], yg[:], mk1_sb[:], None, OP.mult)
        nc.scalar.dma_start(
            a2a_v[1, g], ygm2[:].rearrange("p (j t) -> p j t", t=TSH))

    nc.gpsimd.collective_compute(
        "AllToAll", OP.bypass, replica_groups=G8,
        ins=[a2a_in[:].opt()], outs=[a2a_out[:].opt()])
    es_c.close()

    poolBK = es.enter_context(tc.tile_pool(name="poolBK", bufs=1))
    poolD = es_d.enter_context(tc.tile_pool(name="poolD", bufs=1))
    # yh loads on the Pool queue (already serialized behind the A2A);
    # keeping them off SP lets the tail weight streams (w_out/w1/w2) issue
    # and transfer during the collective window
    yh0 = poolD.tile([128, 16, TSH], BF)
    nc.gpsimd.dma_start(yh0[:], a2a_out[0:DI, :].rearrange("(k p) t -> p k t", p=128))
    yh1 = poolD.tile([128, 16, TSH], BF)
    nc.gpsimd.dma_start(yh1[:], a2a_out[DI:2 * DI, :].rearrange("(k p) t -> p k t", p=128))
    ygf = poolD.tile([128, 16, TSH], BF)
    nc.vector.tensor_tensor(ygf[:], yh0[:], yh1[:], OP.add)
    xsl_sb = poolBK.tile([128, 2, DM], F32)
    nc.sync.dma_start(xsl_sb[:], xsl.rearrange("(h p) m -> p h m", p=128))
    x2 = poolBK.tile([128, 2, DM], F32)
    with tc.tile_pool(name="wop", bufs=16) as wop:
        pso = {}
        with tc.tile_pool(name="pso", bufs=1, space="PSUM") as pso_pool:
            for th in range(2):
                for ms in range(2):
                    pso[(th, ms)] = pso_pool.tile([128, 512], F32,
                                                  tag=f"pso_{th}_{ms}",
                                                  name=f"pso_{th}_{ms}")
            for k in range(16):
                woutk = wop.tile([128, DM], BF, tag="woutk")
                nc.sync.dma_start(woutk[:], w_out[k * 128:(k + 1) * 128, :])
                for th in range(2):
                    for ms in range(2):
                        nc.tensor.matmul(pso[(th, ms)][:],
                                         ygf[:, k, th * 128:(th + 1) * 128],
                                         woutk[:, ms * 512:(ms + 1) * 512],
                                         start=(k == 0), stop=(k == 15))
            for th in range(2):
                for ms in range(2):
                    nc.vector.tensor_tensor(
                        x2[:, th, ms * 512:(ms + 1) * 512], pso[(th, ms)][:],
                        xsl_sb[:, th, ms * 512:(ms + 1) * 512], OP.add)
    es_d.close()

    x2n_dram = dram.tile([TSH, DM], BF)
    x2nT = poolBK.tile([128, 8, TSH], BF)
    for th in range(2):
        st6 = works.tile([128, 12], F32, tag="sm")
        nc.vector.bn_stats(st6[:, 0:6], x2[:, th, 0:512])
        nc.vector.bn_stats(st6[:, 6:12], x2[:, th, 512:1024])
        ag = works.tile([128, 2], F32, tag="sm2")
        nc.vector.bn_aggr(ag[:], st6[:])
        svar = works.tile([128, 1], F32, tag="sm3")
        nc.scalar.activation(svar[:], ag[:, 1:2], AF.Sqrt, bias=eps_sb[:])
        rstd = works.tile([128, 1], F32, tag="sm4")
        nc.vector.reciprocal(rstd[:], svar[:])
        x2n = workb.tile([128, DM], BF, tag="bfw")
        nc.vector.tensor_scalar(x2n[:], x2[:, th, :], ag[:, 0:1], rstd[:],
                                OP.subtract, OP.mult)
        nc.sync.dma_start(x2n_dram[th * 128:(th + 1) * 128, :], x2n[:])
        nc.sync.dma_start_transpose(
            x2nT[:, :, th * 128:(th + 1) * 128],
            x2n_dram[th * 128:(th + 1) * 128, :])

    h1T = poolBK.tile([128, 32, TSH], BF)
    with tc.tile_pool(name="w1p", bufs=16) as w1p, \
         tc.tile_pool(name="pp1", bufs=2, space="PSUM") as pp1:
        for ff in range(32):
            w1f = w1p.tile([128, 8, 128], BF, tag="w1f")
            nc.sync.dma_start(w1f[:], w1[ff])
            ps = pp1.tile([128, TSH], F32, tag="pf1")
            for k in range(8):
                nc.tensor.matmul(ps[:], w1f[:, k, :],
                                 x2nT[:, k, :],
                                 start=(k == 0), stop=(k == 7))
            nc.scalar.activation(h1T[:, ff, :], ps[:], AF.Relu)
    es_e.close()

    with tc.tile_pool(name="pf2", bufs=1, space="PSUM") as pf2, \
         tc.tile_pool(name="w2p", bufs=8) as w2p:
        pss = {}
        for th in range(2):
            for ms in range(2):
                pss[(th, ms)] = pf2.tile([128, 512], F32, tag=f"po2_{th}_{ms}",
                                         name=f"po2_{th}_{ms}")
        for k in range(32):
            w2k = w2p.tile([128, DM], BF, tag="w2k")
            nc.sync.dma_start(w2k[:], w2[k * 128:(k + 1) * 128, :])
            for th in range(2):
                for ms in range(2):
                    nc.tensor.matmul(pss[(th, ms)][:],
                                     h1T[:, k, th * 128:(th + 1) * 128],
                                     w2k[:, ms * 512:(ms + 1) * 512],
                                     start=(k == 0), stop=(k == 31))
        for th in range(2):
            for ms in range(2):
                ot = work.tile([128, 512], F32, tag="f32w")
                nc.vector.tensor_tensor(ot[:], pss[(th, ms)][:],
                                        x2[:, th, ms * 512:(ms + 1) * 512], OP.add)
                nc.sync.dma_start(out_ext[th * 128:(th + 1) * 128,
                                          ms * 512:(ms + 1) * 512], ot[:])

    es.close()


def _prep_core_inputs(inputs):
    x = np.asarray(inputs["x"], np.float32)
    W_in = np.asarray(inputs["W_in"], np.float32)
    conv_w = np.asarray(inputs["conv_w"], np.float32)
    W_xp = np.asarray(inputs["W_xproj"], np.float32)
    W_dt = np.asarray(inputs["W_dt"], np.float32)
    b_dt = np.asarray(inputs["b_dt"], np.float32)
    A_log = np.asarray(inputs["A_log"], np.float32)
    D = np.asarray(inputs["D"], np.float32)
    W_out = np.asarray(inputs["W_out"], np.float32)
    W1 = np.asarray(inputs["W1"], np.float32)
    W2 = np.asarray(inputs["W2"], np.float32)
    ln1_w = np.asarray(inputs["ln1_w"], np.float32)
    ln1_b = np.asarray(inputs["ln1_b"], np.float32)
    ln2_w = np.asarray(inputs["ln2_w"], np.float32)
    ln2_b = np.asarray(inputs["ln2_b"], np.float32)
    b1 = np.asarray(inputs["b1"], np.float32)
    b2 = np.asarray(inputs["b2"], np.float32)
    conv_b = np.asarray(inputs["conv_b"], np.float32)

    W_in = W_in * ln1_w[None, :]
    W1 = W1 * ln2_w[None, :]
    assert np.allclose(ln1_b, 0.0) and np.allclose(ln2_b, 0.0)
    assert np.allclose(b1, 0.0) and np.allclose(b2, 0.0)
    assert np.allclose(conv_b, 0.0)

    A = -np.exp(A_log)

    in_maps = []
    for core in range(8):
        g, r = core // NG, core % NG
        ch = slice(r * CSH, (r + 1) * CSH)
        m = {}
        m["x_in"] = np.ascontiguousarray(x[g]).astype(BF16)
        m["xsl"] = np.ascontiguousarray(x[g][r * TSH:(r + 1) * TSH, :])
        wu = W_in[ch, :]
        wz = W_in[DI + r * CSH: DI + (r + 1) * CSH, :]
        m["w_in"] = np.ascontiguousarray(
            np.concatenate([wu.T, wz.T], axis=1).astype(BF16))
        dg = np.zeros((CSH, DC * 128), np.float32)
        cw = conv_w[ch, :]
        for gg in range(4):
            for c in range(128):
                for k in range(DC):
                    dg[gg * 128 + c, k * 128 + c] = cw[gg * 128 + c, k]
        m["dconv"] = dg.astype(BF16)
        m["w_xp"] = np.ascontiguousarray(W_xp[:, ch].T).astype(BF16)
        m["w_dt"] = np.ascontiguousarray(W_dt[ch, :].T).astype(BF16)
        m["b_dt"] = np.ascontiguousarray(b_dt[ch, None])
        app = np.zeros((128, CSH // 8), np.float32)
        for ct in range(CSH // 8):
            for p in range(128):
                s, d = p // 8, p % 8
                app[p, ct] = A[r * CSH + ct * 8 + d, s]
        m["a_pp"] = app
        dpp = np.zeros((128, 4), np.float32)
        for gg in range(4):
            dpp[:, gg] = D[r * CSH + gg * 128: r * CSH + (gg + 1) * 128]
        m["d_pp"] = dpp
        selm = np.zeros((128, 16 * 128), np.float32)
        seltm = np.zeros((128, 16 * 128), np.float32)
        for q in range(16):
            for p in range(128):
                selm[p, q * 128 + q * 8 + (p % 8)] = 1.0
                seltm[q * 8 + (p % 8), q * 128 + p] = 1.0
        m["sel"] = selm.astype(BF16)
        m["selt"] = seltm.astype(BF16)
        m["w_out"] = np.ascontiguousarray(W_out.T.astype(BF16))
        m["mk0"] = np.full((128, 1), 1.0 if g == 0 else 0.0, np.float32)
        m["mk1"] = np.full((128, 1), 1.0 if g == 1 else 0.0, np.float32)
        w1r = W1.reshape(32, 128, 8, 128)
        w1r = np.transpose(w1r, (0, 3, 2, 1))
        m["w1"] = np.ascontiguousarray(w1r.reshape(32, 128, 8 * 128).astype(BF16))
        m["w2"] = np.ascontiguousarray(W2.T.astype(BF16))
        in_maps.append(m)
    return in_maps


_NC = None


def kernel(**inputs):
    global _NC
    if _NC is None:
        _NC = build_kernel()
    in_maps = _prep_core_inputs(inputs)
    res = bass_utils.run_bass_kernel_spmd(_NC, in_maps, core_ids=list(range(8)))
    out = np.zeros((B, L, DM), np.float32)
    for core in range(8):
        g, r = core // NG, core % NG
        out[g, r * TSH:(r + 1) * TSH, :] = res.results[core]["out"]
    return out
